# revision 9
# baseline (speedup 1.0000x reference)
"""Trainium2 Bass kernel for nn_AttributeBiasLoss.

Reference computation:
    per_node = mean(sigmoid(predictions), axis=1)            # [B]
    for each attribute a: group per_node by attr_vals[:, a] (V=16 values)
    means[a, v] = mean of per_node over group (a, v)
    loss = sum over attrs of pairwise squared diffs of present group means
           / number of comparisons

Kernel strategy (data-parallel over 8 cores, batch-sharded):
  One-hot + TensorEngine reduction with the sigmoids as the stationary
  operand.  Per column chunk:
    - ACT: sig9[p, c, 0:8] = sigmoid(pred) bf16 (slot 8 preset to 1.0)
    - DVE: H[p, c, v*8+a] = (x[p,c,a] == v) bf16 for v < 15 only
      (16 tensor_scalar is_equal passes -> 15; 4x perf mode).  Column 120
      of each c is a preset ones-column (gives column sums of sig = total
      T for the v=15 reconstruction), column 121 is zero padding so the
      per-column stride stays even (4x alignment).
    - PE : per group of W=4 columns, lhsT = sig9 (36 cols), rhs = H
      (488 cols, bf16): PSUM[36, 488] accumulates all 490 groups.
      Diagonal blocks (rows 9w..9w+8, col block w) hold per-(a,v<15)
      sums weighted by each sigmoid (rows 9w+d) and counts (row 9w+8).
  v=15 stats are reconstructed in the epilogue: S15 = T - sum_v<15 S,
  n15 = B - sum_v<15 n.  Pads use x=16 (one-hot zero) and pred=-20
  (sigmoid ~ 0) so they drop out of every statistic.
  A tiny fold-matmul collapses PSUM[36,488] -> [2,122] before the
  AllReduce (976B payload); per-core epilogue computes the loss via
  sum_{i<j}(m_i-m_j)^2 = V*sum m^2 - (sum m)^2 (all 128 groups are
  provably non-empty at B=2M).
"""

import sys

sys.path.insert(0, "/opt/trn_rl_repo")

from contextlib import ExitStack

import numpy as np

import concourse.bacc as bacc
import concourse.bass as bass
import concourse.mybir as mybir
import concourse.tile as tile
from concourse import bass_utils
from concourse._compat import with_exitstack

F32 = mybir.dt.float32
BF16 = mybir.dt.bfloat16
AF = mybir.ActivationFunctionType
OP = mybir.AluOpType
AX = mybir.AxisListType

# Problem constants (hardcoded per harness contract).
B, D, A, V = 2_000_000, 8, 8, 16
NCORES = 8
ROWS_PER_CORE = B // NCORES  # 250_000

PAD_ATTR = 16  # out-of-range bucket: one-hot all-zero, excluded from stats
PAD_PRED = -20.0  # sigmoid(-20) ~ 2e-9: excluded from T

CP = 1960  # columns per partition
ROWS_PAD = 128 * CP  # 250_880
CS = 96  # main chunk size (columns)
SUBS = [16, 80] + [CS] * 19 + [CP - 20 * CS]  # 1960 total
W = 4  # data columns per matmul group
NW = 9  # weight cols per data column: 8 sigmoids + ones
KCOL = 122  # rhs cols per data column: 15*8 one-hot + ones + pad
NCMP = float(A * V * (V - 1) // 2)  # 960 comparisons (all groups present)


@with_exitstack
def emit_kernel(
    ctx: ExitStack,
    tc: tile.TileContext,
    pred_d,  # DRAM [ROWS_PAD, D] bf16 (host-cast)
    attr_d,  # DRAM [ROWS_PAD, A] bf16 (host-cast)
    wfold_d,  # DRAM [36, 8] f32 fold-weight constant
    loss_d,  # DRAM [1, 1] f32
    dbg_d=None,
    n_cores=NCORES,
):
    nc = tc.nc

    io = ctx.enter_context(tc.tile_pool(name="io", bufs=3))
    hp = ctx.enter_context(tc.tile_pool(name="h", bufs=1))
    smallp = ctx.enter_context(tc.tile_pool(name="small", bufs=1))
    psump = ctx.enter_context(tc.tile_pool(name="ps", bufs=1, space="PSUM"))
    dramp = ctx.enter_context(tc.tile_pool(name="dram", bufs=1, space="DRAM"))

    # Warm up the collective engine early (channel setup dominates the first
    # collective); no data dependencies so it overlaps the main compute.
    NFLATW = 64
    warm_in = dramp.tile([1, NFLATW], F32, name="warm_in")
    warm_out = dramp.tile([1, NFLATW], F32, name="warm_out")
    warm_s = smallp.tile([1, NFLATW], F32, name="warm_s")
    nc.vector.memset(warm_s[:], 0.0)
    nc.sync.dma_start(warm_in[:], warm_s[:])
    nc.gpsimd.collective_compute(
        "AllReduce",
        OP.add,
        replica_groups=[list(range(n_cores))],
        ins=[warm_in.opt()],
        outs=[warm_out.opt()],
    )

    # Double-buffered one-hot / sigmoid tiles.  Ones/pad planes are set once
    # (buffer 1's planes are deferred into the loop to shorten the prologue).
    hbuf = [hp.tile([128, CS * KCOL], BF16, name=f"h{i}") for i in range(2)]
    sbuf = [hp.tile([128, CS * NW], BF16, name=f"s{i}") for i in range(2)]

    def set_planes(i):
        hv = hbuf[i].rearrange("p (c k) -> p c k", k=KCOL)
        nc.vector.memset(hv[:, :, 120:121], 1.0)
        nc.vector.memset(hv[:, :, 121:122], 0.0)
        sv = sbuf[i].rearrange("p (c k) -> p c k", k=NW)
        nc.vector.memset(sv[:, :, 8:9], 1.0)

    set_planes(0)

    # Warm the PE's HAM clock gate with dummy matmuls while DMA/DVE fill the
    # first chunk, so the real matmuls start at 2.4 GHz instead of 1.2.
    scratch = hp.tile([128, 512], BF16, name="scratch")
    nc.vector.memset(scratch[:], 0.0)
    warm_ps = psump.tile([128, 512], F32)
    for i in range(8):
        nc.tensor.matmul(
            warm_ps[:, :],
            lhsT=scratch[:, 0:128],
            rhs=scratch[:, :],
            start=(i == 0),
            stop=(i == 7),
        )

    # Fold weights: out2[2w, :] += (1/8)*sum_d psum[9w+d, wblock];
    #               out2[2w+1, :] += psum[9w+8, wblock].  Host-provided
    # constant (engines cannot memset at a partition offset).
    wfold = smallp.tile([36, 8], F32, name="wfold")
    nc.sync.dma_start(wfold[:], wfold_d[:])

    pred_v = pred_d.rearrange("(p c) d -> p (c d)", p=128)
    attr_v = attr_d.rearrange("(p c) a -> p (c a)", p=128)

    stats_ps = psump.tile([36, W * KCOL], F32)  # [36, 488]

    ngrp_total = CP // W  # 490
    gdone = 0
    pos = 0
    for si, cs in enumerate(SUBS):
        x_t = io.tile([128, cs * A], BF16, tag="attr")
        nc.sync.dma_start(x_t[:], attr_v[:, pos * A : (pos + cs) * A])
        p_t = io.tile([128, cs * D], BF16, tag="pred")
        nc.sync.dma_start(p_t[:], pred_v[:, pos * D : (pos + cs) * D])

        h = hbuf[si % 2]
        hv = h.rearrange("p (c k) -> p c k", k=KCOL)
        sg = sbuf[si % 2]
        sv = sg.rearrange("p (c k) -> p c k", k=NW)

        x_ca = x_t.rearrange("p (c a) -> p c a", a=A)
        # One-hot (4x perf mode: 2-byte dtype, packed inner dim, SBUF).
        for v in range(V - 1):
            nc.vector.tensor_scalar(
                out=hv[:, :cs, v * A : (v + 1) * A],
                in0=x_ca,
                scalar1=float(v),
                scalar2=None,
                op0=OP.is_equal,
            )

        with nc.allow_low_precision(reason="bf16 sigmoid adds ~1e-6 to means"):
            nc.scalar.activation(
                sv[:, :cs, 0:8],
                p_t.rearrange("p (c d) -> p c d", d=D),
                AF.Sigmoid,
            )

        if si == 0:
            set_planes(1)
        for g in range(cs // W):
            c0 = g * W
            nc.tensor.matmul(
                stats_ps[:, :],
                lhsT=sg[:, c0 * NW : (c0 + W) * NW],
                rhs=h[:, c0 * KCOL : (c0 + W) * KCOL],
                start=(gdone == 0),
                stop=(gdone == ngrp_total - 1),
            )
            gdone += 1
        pos += cs

    # ---- tail: fold [36, 488] -> [2, 122], AllReduce 976B, epilogue ----
    sbst = smallp.tile([36, W * KCOL], F32, name="sbst")
    nc.vector.tensor_copy(sbst[:], stats_ps[:])
    out2 = psump.tile([2, KCOL], F32)
    for w in range(W):
        nc.tensor.matmul(
            out2[:, :],
            lhsT=wfold[:, 2 * w : 2 * w + 2],
            rhs=sbst[:, w * KCOL : (w + 1) * KCOL],
            start=(w == 0),
            stop=(w == W - 1),
        )
    g2 = smallp.tile([2, KCOL], F32, name="g2")
    nc.vector.tensor_copy(g2[:], out2[:])

    NFLAT = 2 * KCOL  # 244
    cc_in = dramp.tile([1, NFLAT], F32, name="cc_in")
    cc_out = dramp.tile([1, NFLAT], F32, name="cc_out")
    nc.sync.dma_start(cc_in[:], g2[:])
    nc.gpsimd.collective_compute(
        "AllReduce",
        OP.add,
        replica_groups=[list(range(n_cores))],
        ins=[cc_in.opt()],
        outs=[cc_out.opt()],
    )
    gf = smallp.tile([1, NFLAT], F32, name="gf")
    nc.sync.dma_start(gf[:], cc_out[:])
    if dbg_d is not None:
        nc.sync.dma_start(dbg_d[:], gf[:])

    # ---------------- epilogue (tiny, partition 0, redundant per core) ------
    # Centered variance identity: sum_{i<j}(m_i-m_j)^2 = V*sum_v (m_v-mu)^2.
    # The uncentered V*sum m^2 - (sum m)^2 is catastrophically cancelled in
    # f32 (~64 - 64 with a 1e-5 signal).
    ep = ctx.enter_context(tc.tile_pool(name="ep", bufs=1))
    VV = V - 1  # stored values per attribute

    # permute stats to a-major so mu broadcasts over a trailing v axis
    Sp = ep.tile([1, A * VV], F32, name="Sp")
    nc.vector.tensor_copy(
        Sp[:].rearrange("p (a v) -> p a v", a=A),
        gf[:, 0:120].rearrange("p (v a) -> p a v", v=VV),
    )
    Np = ep.tile([1, A * VV], F32, name="Np")
    nc.vector.tensor_copy(
        Np[:].rearrange("p (a v) -> p a v", a=A),
        gf[:, KCOL : KCOL + 120].rearrange("p (v a) -> p a v", v=VV),
    )
    T_ap = gf[:, 120:121]  # total sum of per_node (all rows)

    sumS = ep.tile([1, A], F32, name="sumS")
    nc.vector.tensor_reduce(
        sumS[:], Sp[:].rearrange("p (a v) -> p a v", a=A), op=OP.add, axis=AX.X
    )
    S15 = ep.tile([1, A], F32, name="S15")
    nc.vector.tensor_scalar(
        out=S15[:], in0=sumS[:], scalar1=T_ap, scalar2=-1.0,
        op0=OP.subtract, op1=OP.mult,
    )
    sumN = ep.tile([1, A], F32, name="sumN")
    nc.vector.tensor_reduce(
        sumN[:], Np[:].rearrange("p (a v) -> p a v", a=A), op=OP.add, axis=AX.X
    )
    n15 = ep.tile([1, A], F32, name="n15")
    nc.vector.tensor_scalar(
        out=n15[:], in0=sumN[:], scalar1=-1.0, scalar2=float(B),
        op0=OP.mult, op1=OP.add,
    )

    rn = ep.tile([1, A * VV], F32, name="rn")
    nc.vector.reciprocal(rn[:], Np[:])
    m = ep.tile([1, A * VV], F32, name="m")
    nc.vector.tensor_tensor(out=m[:], in0=Sp[:], in1=rn[:], op=OP.mult)
    rn15 = ep.tile([1, A], F32, name="rn15")
    nc.vector.reciprocal(rn15[:], n15[:])
    m15 = ep.tile([1, A], F32, name="m15")
    nc.vector.tensor_tensor(out=m15[:], in0=S15[:], in1=rn15[:], op=OP.mult)

    sm = ep.tile([1, A], F32, name="sm")
    nc.vector.tensor_reduce(
        sm[:], m[:].rearrange("p (a v) -> p a v", a=A), op=OP.add, axis=AX.X
    )
    nc.vector.tensor_tensor(out=sm[:], in0=sm[:], in1=m15[:], op=OP.add)
    mu = ep.tile([1, A], F32, name="mu")
    nc.vector.tensor_scalar(
        out=mu[:], in0=sm[:], scalar1=1.0 / V, scalar2=None, op0=OP.mult
    )

    d = ep.tile([1, A * VV], F32, name="d").rearrange("p (a v) -> p a v", a=A)
    nc.vector.tensor_tensor(
        out=d,
        in0=m[:].rearrange("p (a v) -> p a v", a=A),
        in1=mu[:].broadcast_to([1, A, VV]),
        op=OP.subtract,
    )
    d15 = ep.tile([1, A], F32, name="d15")
    nc.vector.tensor_tensor(out=d15[:], in0=m15[:], in1=mu[:], op=OP.subtract)

    d2 = ep.tile([1, A * VV], F32, name="d2")
    nc.vector.tensor_tensor(
        out=d2[:].rearrange("p (a v) -> p a v", a=A), in0=d, in1=d, op=OP.mult
    )
    q = ep.tile([1, A], F32, name="q")
    nc.vector.tensor_reduce(
        q[:], d2[:].rearrange("p (a v) -> p a v", a=A), op=OP.add, axis=AX.X
    )
    d15b = ep.tile([1, A], F32, name="d15b")
    nc.vector.tensor_tensor(out=d15b[:], in0=d15[:], in1=d15[:], op=OP.mult)
    nc.vector.tensor_tensor(out=q[:], in0=q[:], in1=d15b[:], op=OP.add)

    # loss = sum_a V * q_a / NCMP  (= sum_a q_a / 60)
    tot = ep.tile([1, 1], F32, name="tot")
    nc.vector.tensor_reduce(tot[:], q[:], op=OP.add, axis=AX.X)
    nc.vector.tensor_scalar(
        out=tot[:], in0=tot[:], scalar1=float(V) / NCMP, scalar2=None, op0=OP.mult
    )

    nc.sync.dma_start(loss_d[:], tot[:])


def build(n_cores=NCORES, debug_out=False):
    nc = bacc.Bacc(
        "TRN2", target_bir_lowering=False, debug=False, num_devices=n_cores
    )
    pred_d = nc.dram_tensor("pred", [ROWS_PAD, D], BF16, kind="ExternalInput").ap()
    attr_d = nc.dram_tensor("attr", [ROWS_PAD, A], BF16, kind="ExternalInput").ap()
    wfold_d = nc.dram_tensor("wfold", [36, 8], F32, kind="ExternalInput").ap()
    loss_d = nc.dram_tensor("loss", [1, 1], F32, kind="ExternalOutput").ap()
    dbg_d = nc.dram_tensor("dbg", [1, 244], F32, kind="ExternalOutput").ap() if debug_out else None
    with tile.TileContext(nc) as tc:
        emit_kernel(tc, pred_d, attr_d, wfold_d, loss_d, dbg_d=dbg_d, n_cores=n_cores)
    nc.compile()
    return nc


try:
    import ml_dtypes

    _BF16_NP = ml_dtypes.bfloat16
except Exception:  # pragma: no cover
    import jax.numpy as jnp

    _BF16_NP = jnp.bfloat16


def _wfold_const():
    wf = np.zeros((36, 8), np.float32)
    for w in range(W):
        wf[9 * w : 9 * w + 8, 2 * w] = 0.125
        wf[9 * w + 8, 2 * w + 1] = 1.0
    return wf


def shard_inputs(predictions, attr_vals, n_cores=NCORES, rows_pad=ROWS_PAD):
    rows = predictions.shape[0] // n_cores
    wf = _wfold_const()
    in_maps = []
    for c in range(n_cores):
        p = predictions[c * rows : (c + 1) * rows]
        a = attr_vals[c * rows : (c + 1) * rows]
        pad = rows_pad - rows
        if pad:
            p = np.concatenate(
                [p, np.full((pad, D), PAD_PRED, np.float32)], axis=0
            )
            a = np.concatenate([a, np.full((pad, A), PAD_ATTR, np.int32)], axis=0)
        p16 = p.astype(_BF16_NP)
        a16 = a.astype(np.float32).astype(_BF16_NP)
        in_maps.append(
            {
                "pred": np.ascontiguousarray(p16),
                "attr": np.ascontiguousarray(a16),
                "wfold": wf,
            }
        )
    return in_maps


_NC_CACHE = {}


def kernel(predictions: np.ndarray, attr_vals: np.ndarray) -> np.ndarray:
    predictions = np.asarray(predictions, np.float32)
    attr_vals = np.asarray(attr_vals, np.int32)
    if "nc" not in _NC_CACHE:
        _NC_CACHE["nc"] = build()
    nc = _NC_CACHE["nc"]
    in_maps = shard_inputs(predictions, attr_vals)
    res = bass_utils.run_bass_kernel_spmd(nc, in_maps, list(range(NCORES)))
    return np.float32(res.results[0]["loss"][0, 0])


# revision 10
# speedup vs baseline: 1.0136x; 1.0136x over previous
"""Trainium2 Bass kernel for nn_AttributeBiasLoss.

Reference computation:
    per_node = mean(sigmoid(predictions), axis=1)            # [B]
    for each attribute a: group per_node by attr_vals[:, a] (V=16 values)
    means[a, v] = mean of per_node over group (a, v)
    loss = sum over attrs of pairwise squared diffs of present group means
           / number of comparisons

Kernel strategy (data-parallel over 8 cores, batch-sharded):
  One-hot + TensorEngine reduction with the sigmoids as the stationary
  operand.  Per column chunk:
    - ACT: sig9[p, c, 0:8] = sigmoid(pred) bf16 (slot 8 preset to 1.0)
    - DVE: H[p, c, v*8+a] = (x[p,c,a] == v) bf16 for v < 15 only
      (16 tensor_scalar is_equal passes -> 15; 4x perf mode).  Column 120
      of each c is a preset ones-column (gives column sums of sig = total
      T for the v=15 reconstruction), column 121 is zero padding so the
      per-column stride stays even (4x alignment).
    - PE : per group of W=4 columns, lhsT = sig9 (36 cols), rhs = H
      (488 cols, bf16): PSUM[36, 488] accumulates all 490 groups.
      Diagonal blocks (rows 9w..9w+8, col block w) hold per-(a,v<15)
      sums weighted by each sigmoid (rows 9w+d) and counts (row 9w+8).
  v=15 stats are reconstructed in the epilogue: S15 = T - sum_v<15 S,
  n15 = B - sum_v<15 n.  Pads use x=16 (one-hot zero) and pred=-20
  (sigmoid ~ 0) so they drop out of every statistic.
  A tiny fold-matmul collapses PSUM[36,488] -> [2,122] before the
  AllReduce (976B payload); per-core epilogue computes the loss via
  sum_{i<j}(m_i-m_j)^2 = V*sum m^2 - (sum m)^2 (all 128 groups are
  provably non-empty at B=2M).
"""

import sys

sys.path.insert(0, "/opt/trn_rl_repo")

from contextlib import ExitStack

import numpy as np

import concourse.bacc as bacc
import concourse.bass as bass
import concourse.mybir as mybir
import concourse.tile as tile
from concourse import bass_utils
from concourse._compat import with_exitstack

F32 = mybir.dt.float32
BF16 = mybir.dt.bfloat16
AF = mybir.ActivationFunctionType
OP = mybir.AluOpType
AX = mybir.AxisListType

# Problem constants (hardcoded per harness contract).
B, D, A, V = 2_000_000, 8, 8, 16
NCORES = 8
ROWS_PER_CORE = B // NCORES  # 250_000

PAD_ATTR = 16  # out-of-range bucket: one-hot all-zero, excluded from stats
PAD_PRED = -20.0  # sigmoid(-20) ~ 2e-9: excluded from T

CP = 1960  # columns per partition
ROWS_PAD = 128 * CP  # 250_880
CS = 96  # main chunk size (columns)
SUBS = [48, 48] + [CS] * 19 + [CP - 20 * CS]  # 1960 total
W = 4  # data columns per matmul group
NW = 9  # weight cols per data column: 8 sigmoids + ones
KCOL = 122  # rhs cols per data column: 15*8 one-hot + ones + pad
NCMP = float(A * V * (V - 1) // 2)  # 960 comparisons (all groups present)


@with_exitstack
def emit_kernel(
    ctx: ExitStack,
    tc: tile.TileContext,
    pred_d,  # DRAM [ROWS_PAD, D] bf16 (host-cast)
    attr_d,  # DRAM [ROWS_PAD, A] bf16 (host-cast)
    wfold_d,  # DRAM [36, 8] f32 fold-weight constant
    loss_d,  # DRAM [1, 1] f32
    dbg_d=None,
    n_cores=NCORES,
):
    nc = tc.nc

    io = ctx.enter_context(tc.tile_pool(name="io", bufs=3))
    hp = ctx.enter_context(tc.tile_pool(name="h", bufs=1))
    smallp = ctx.enter_context(tc.tile_pool(name="small", bufs=1))
    psump = ctx.enter_context(tc.tile_pool(name="ps", bufs=1, space="PSUM"))
    dramp = ctx.enter_context(tc.tile_pool(name="dram", bufs=1, space="DRAM"))

    # Warm up the collective engine early (channel setup dominates the first
    # collective); no data dependencies so it overlaps the main compute.
    NFLATW = 64
    warm_in = dramp.tile([1, NFLATW], F32, name="warm_in")
    warm_out = dramp.tile([1, NFLATW], F32, name="warm_out")
    warm_s = smallp.tile([1, NFLATW], F32, name="warm_s")
    nc.vector.memset(warm_s[:], 0.0)
    nc.sync.dma_start(warm_in[:], warm_s[:])
    nc.gpsimd.collective_compute(
        "AllReduce",
        OP.add,
        replica_groups=[list(range(n_cores))],
        ins=[warm_in.opt()],
        outs=[warm_out.opt()],
    )

    # Double-buffered one-hot / sigmoid tiles.  Ones/pad planes are set once
    # (buffer 1's planes are deferred into the loop to shorten the prologue).
    hbuf = [hp.tile([128, CS * KCOL], BF16, name=f"h{i}") for i in range(2)]
    sbuf = [hp.tile([128, CS * NW], BF16, name=f"s{i}") for i in range(2)]

    def set_planes(i):
        hv = hbuf[i].rearrange("p (c k) -> p c k", k=KCOL)
        nc.vector.memset(hv[:, :, 120:121], 1.0)
        nc.vector.memset(hv[:, :, 121:122], 0.0)
        sv = sbuf[i].rearrange("p (c k) -> p c k", k=NW)
        nc.vector.memset(sv[:, :, 8:9], 1.0)

    set_planes(0)

    # Warm the PE's HAM clock gate with dummy matmuls while DMA/DVE fill the
    # first chunk, so the real matmuls start at 2.4 GHz instead of 1.2.
    scratch = hp.tile([128, 512], BF16, name="scratch")
    nc.vector.memset(scratch[:], 0.0)
    warm_ps = psump.tile([128, 512], F32)
    for i in range(8):
        nc.tensor.matmul(
            warm_ps[:, :],
            lhsT=scratch[:, 0:128],
            rhs=scratch[:, :],
            start=(i == 0),
            stop=(i == 7),
        )

    # Fold weights: out2[2w, :] += (1/8)*sum_d psum[9w+d, wblock];
    #               out2[2w+1, :] += psum[9w+8, wblock].  Host-provided
    # constant (engines cannot memset at a partition offset).
    wfold = smallp.tile([36, 8], F32, name="wfold")
    nc.sync.dma_start(wfold[:], wfold_d[:])

    pred_v = pred_d.rearrange("(p c) d -> p (c d)", p=128)
    attr_v = attr_d.rearrange("(p c) a -> p (c a)", p=128)

    stats_ps = psump.tile([36, W * KCOL], F32)  # [36, 488]

    ngrp_total = CP // W  # 490
    gdone = 0
    pos = 0
    for si, cs in enumerate(SUBS):
        x_t = io.tile([128, cs * A], BF16, tag="attr")
        nc.sync.dma_start(x_t[:], attr_v[:, pos * A : (pos + cs) * A])
        p_t = io.tile([128, cs * D], BF16, tag="pred")
        nc.sync.dma_start(p_t[:], pred_v[:, pos * D : (pos + cs) * D])

        h = hbuf[si % 2]
        hv = h.rearrange("p (c k) -> p c k", k=KCOL)
        sg = sbuf[si % 2]
        sv = sg.rearrange("p (c k) -> p c k", k=NW)

        x_ca = x_t.rearrange("p (c a) -> p c a", a=A)
        # One-hot (4x perf mode: 2-byte dtype, packed inner dim, SBUF).
        for v in range(V - 1):
            nc.vector.tensor_scalar(
                out=hv[:, :cs, v * A : (v + 1) * A],
                in0=x_ca,
                scalar1=float(v),
                scalar2=None,
                op0=OP.is_equal,
            )

        with nc.allow_low_precision(reason="bf16 sigmoid adds ~1e-6 to means"):
            nc.scalar.activation(
                sv[:, :cs, 0:8],
                p_t.rearrange("p (c d) -> p c d", d=D),
                AF.Sigmoid,
            )

        if si == 0:
            set_planes(1)
        for g in range(cs // W):
            c0 = g * W
            nc.tensor.matmul(
                stats_ps[:, :],
                lhsT=sg[:, c0 * NW : (c0 + W) * NW],
                rhs=h[:, c0 * KCOL : (c0 + W) * KCOL],
                start=(gdone == 0),
                stop=(gdone == ngrp_total - 1),
            )
            gdone += 1
        pos += cs

    # ---- tail: fold [36, 488] -> [2, 122], AllReduce 976B, epilogue ----
    sbst = smallp.tile([36, W * KCOL], F32, name="sbst")
    nc.vector.tensor_copy(sbst[:], stats_ps[:])
    out2 = psump.tile([2, KCOL], F32)
    for w in range(W):
        nc.tensor.matmul(
            out2[:, :],
            lhsT=wfold[:, 2 * w : 2 * w + 2],
            rhs=sbst[:, w * KCOL : (w + 1) * KCOL],
            start=(w == 0),
            stop=(w == W - 1),
        )
    g2 = smallp.tile([2, KCOL], F32, name="g2")
    nc.vector.tensor_copy(g2[:], out2[:])

    NFLAT = 2 * KCOL  # 244
    cc_in = dramp.tile([1, NFLAT], F32, name="cc_in")
    cc_out = dramp.tile([1, NFLAT * n_cores], F32, name="cc_out")
    nc.sync.dma_start(cc_in[:], g2[:])
    nc.gpsimd.collective_compute(
        "AllGather",
        OP.bypass,
        replica_groups=[list(range(n_cores))],
        ins=[cc_in.opt()],
        outs=[cc_out.opt()],
    )
    gfa = smallp.tile([1, NFLAT * n_cores], F32, name="gfa")
    nc.sync.dma_start(gfa[:], cc_out[:])
    # fold the 8 per-core payloads (sum over the core axis)
    gf = smallp.tile([1, NFLAT], F32, name="gf")
    nc.vector.tensor_reduce(
        gf[:].rearrange("p (o q) -> p q o", o=1),
        gfa[:].rearrange("p (k q) -> p q k", k=n_cores),
        op=OP.add,
        axis=AX.X,
    )
    if dbg_d is not None:
        nc.sync.dma_start(dbg_d[:], gf[:])

    # ---------------- epilogue (tiny, partition 0, redundant per core) ------
    # Centered variance identity: sum_{i<j}(m_i-m_j)^2 = V*sum_v (m_v-mu)^2.
    # The uncentered V*sum m^2 - (sum m)^2 is catastrophically cancelled in
    # f32 (~64 - 64 with a 1e-5 signal).
    ep = ctx.enter_context(tc.tile_pool(name="ep", bufs=1))
    VV = V - 1  # stored values per attribute

    # permute stats to a-major so mu broadcasts over a trailing v axis
    Sp = ep.tile([1, A * VV], F32, name="Sp")
    nc.vector.tensor_copy(
        Sp[:].rearrange("p (a v) -> p a v", a=A),
        gf[:, 0:120].rearrange("p (v a) -> p a v", v=VV),
    )
    Np = ep.tile([1, A * VV], F32, name="Np")
    nc.vector.tensor_copy(
        Np[:].rearrange("p (a v) -> p a v", a=A),
        gf[:, KCOL : KCOL + 120].rearrange("p (v a) -> p a v", v=VV),
    )
    T_ap = gf[:, 120:121]  # total sum of per_node (all rows)

    sumS = ep.tile([1, A], F32, name="sumS")
    nc.vector.tensor_reduce(
        sumS[:], Sp[:].rearrange("p (a v) -> p a v", a=A), op=OP.add, axis=AX.X
    )
    S15 = ep.tile([1, A], F32, name="S15")
    nc.vector.tensor_scalar(
        out=S15[:], in0=sumS[:], scalar1=T_ap, scalar2=-1.0,
        op0=OP.subtract, op1=OP.mult,
    )
    sumN = ep.tile([1, A], F32, name="sumN")
    nc.vector.tensor_reduce(
        sumN[:], Np[:].rearrange("p (a v) -> p a v", a=A), op=OP.add, axis=AX.X
    )
    n15 = ep.tile([1, A], F32, name="n15")
    nc.vector.tensor_scalar(
        out=n15[:], in0=sumN[:], scalar1=-1.0, scalar2=float(B),
        op0=OP.mult, op1=OP.add,
    )

    rn = ep.tile([1, A * VV], F32, name="rn")
    nc.vector.reciprocal(rn[:], Np[:])
    m = ep.tile([1, A * VV], F32, name="m")
    nc.vector.tensor_tensor(out=m[:], in0=Sp[:], in1=rn[:], op=OP.mult)
    rn15 = ep.tile([1, A], F32, name="rn15")
    nc.vector.reciprocal(rn15[:], n15[:])
    m15 = ep.tile([1, A], F32, name="m15")
    nc.vector.tensor_tensor(out=m15[:], in0=S15[:], in1=rn15[:], op=OP.mult)

    sm = ep.tile([1, A], F32, name="sm")
    nc.vector.tensor_reduce(
        sm[:], m[:].rearrange("p (a v) -> p a v", a=A), op=OP.add, axis=AX.X
    )
    nc.vector.tensor_tensor(out=sm[:], in0=sm[:], in1=m15[:], op=OP.add)
    mu = ep.tile([1, A], F32, name="mu")
    nc.vector.tensor_scalar(
        out=mu[:], in0=sm[:], scalar1=1.0 / V, scalar2=None, op0=OP.mult
    )

    d = ep.tile([1, A * VV], F32, name="d").rearrange("p (a v) -> p a v", a=A)
    nc.vector.tensor_tensor(
        out=d,
        in0=m[:].rearrange("p (a v) -> p a v", a=A),
        in1=mu[:].broadcast_to([1, A, VV]),
        op=OP.subtract,
    )
    d15 = ep.tile([1, A], F32, name="d15")
    nc.vector.tensor_tensor(out=d15[:], in0=m15[:], in1=mu[:], op=OP.subtract)

    d2 = ep.tile([1, A * VV], F32, name="d2")
    nc.vector.tensor_tensor(
        out=d2[:].rearrange("p (a v) -> p a v", a=A), in0=d, in1=d, op=OP.mult
    )
    q = ep.tile([1, A], F32, name="q")
    nc.vector.tensor_reduce(
        q[:], d2[:].rearrange("p (a v) -> p a v", a=A), op=OP.add, axis=AX.X
    )
    d15b = ep.tile([1, A], F32, name="d15b")
    nc.vector.tensor_tensor(out=d15b[:], in0=d15[:], in1=d15[:], op=OP.mult)
    nc.vector.tensor_tensor(out=q[:], in0=q[:], in1=d15b[:], op=OP.add)

    # loss = sum_a V * q_a / NCMP  (= sum_a q_a / 60)
    tot = ep.tile([1, 1], F32, name="tot")
    nc.vector.tensor_reduce(tot[:], q[:], op=OP.add, axis=AX.X)
    nc.vector.tensor_scalar(
        out=tot[:], in0=tot[:], scalar1=float(V) / NCMP, scalar2=None, op0=OP.mult
    )

    nc.sync.dma_start(loss_d[:], tot[:])


def build(n_cores=NCORES, debug_out=False):
    nc = bacc.Bacc(
        "TRN2", target_bir_lowering=False, debug=False, num_devices=n_cores
    )
    pred_d = nc.dram_tensor("pred", [ROWS_PAD, D], BF16, kind="ExternalInput").ap()
    attr_d = nc.dram_tensor("attr", [ROWS_PAD, A], BF16, kind="ExternalInput").ap()
    wfold_d = nc.dram_tensor("wfold", [36, 8], F32, kind="ExternalInput").ap()
    loss_d = nc.dram_tensor("loss", [1, 1], F32, kind="ExternalOutput").ap()
    dbg_d = nc.dram_tensor("dbg", [1, 244], F32, kind="ExternalOutput").ap() if debug_out else None
    with tile.TileContext(nc) as tc:
        emit_kernel(tc, pred_d, attr_d, wfold_d, loss_d, dbg_d=dbg_d, n_cores=n_cores)
    nc.compile()
    return nc


try:
    import ml_dtypes

    _BF16_NP = ml_dtypes.bfloat16
except Exception:  # pragma: no cover
    import jax.numpy as jnp

    _BF16_NP = jnp.bfloat16


def _wfold_const():
    wf = np.zeros((36, 8), np.float32)
    for w in range(W):
        wf[9 * w : 9 * w + 8, 2 * w] = 0.125
        wf[9 * w + 8, 2 * w + 1] = 1.0
    return wf


def shard_inputs(predictions, attr_vals, n_cores=NCORES, rows_pad=ROWS_PAD):
    rows = predictions.shape[0] // n_cores
    wf = _wfold_const()
    in_maps = []
    for c in range(n_cores):
        p = predictions[c * rows : (c + 1) * rows]
        a = attr_vals[c * rows : (c + 1) * rows]
        pad = rows_pad - rows
        if pad:
            p = np.concatenate(
                [p, np.full((pad, D), PAD_PRED, np.float32)], axis=0
            )
            a = np.concatenate([a, np.full((pad, A), PAD_ATTR, np.int32)], axis=0)
        p16 = p.astype(_BF16_NP)
        a16 = a.astype(np.float32).astype(_BF16_NP)
        in_maps.append(
            {
                "pred": np.ascontiguousarray(p16),
                "attr": np.ascontiguousarray(a16),
                "wfold": wf,
            }
        )
    return in_maps


_NC_CACHE = {}


def kernel(predictions: np.ndarray, attr_vals: np.ndarray) -> np.ndarray:
    predictions = np.asarray(predictions, np.float32)
    attr_vals = np.asarray(attr_vals, np.int32)
    if "nc" not in _NC_CACHE:
        _NC_CACHE["nc"] = build()
    nc = _NC_CACHE["nc"]
    in_maps = shard_inputs(predictions, attr_vals)
    res = bass_utils.run_bass_kernel_spmd(nc, in_maps, list(range(NCORES)))
    return np.float32(res.results[0]["loss"][0, 0])
